# revision 10
# baseline (speedup 1.0000x reference)
"""Block cross-attention Trainium2 kernel (Bass/Tile), 8-core SPMD.

Reference computation (see problem statement):
  pooled = mean-pool x over blocks of 16 tokens        [B, nb, D]
  q = pooled @ Wq (16 heads), k/v = enc @ Wk/Wv (4 kv heads, GQA)
  p = softmax(q k^T * scale + mask)                    per kv-head group
  o = p @ v ; out = repeat(o @ Wo, 16 tokens/block)    [B, L, D]

Sharding: 8 cores = (batch b in {0,1}) x (block-range r in {0..3}).
Each core owns 128 query blocks (2048 tokens) of one batch and computes
ALL heads for them, so the output projection finishes on-device with no
cross-core reduction.  The kv projection (full S for all 4 kv heads) is
recomputed per core; softmax work is perfectly sharded.

Device pipeline per core:
  pool:   x-slice [2048, D] -> pooled-sum [128 blocks, D]  (DVE tree add;
          the /16 is folded into the exp scale)
  q:      pooledT (PE transpose) @ Wq -> q [128, 1024] -> per-head PE
          transposes -> qT_g [64, 4*128] per kv-group
  kv:     per 512-row strip of enc: PE transpose -> encT, kvT_g[128, 512]
          = (Wk_g|Wv_g)^T @ encT; then per 128-col chunk: sT = kT^T @ qT_g,
          p = exp(scale*sT + maskbias) on ACT, v_aug = (v|1) via PE
          transpose, oT_g += v_aug^T @ p  (ones column accumulates the
          softmax denominator for free)
  norm:   oTn_g = oT_g[0:64] * bcast(1/oT_g[64])  (PE broadcast)
  outp:   out_blocks [128, D] = sum_h oTn^T @ Wo_h ; broadcast each block
          row to its 16 tokens on the DMA out.
"""

import numpy as np

import concourse.bass as bass
import concourse.tile as tile
from concourse import bacc, mybir
from concourse.bass_utils import run_bass_kernel_spmd
from concourse.masks import make_identity

F32 = mybir.dt.float32
F32R = mybir.dt.float32r
I32 = mybir.dt.int32
EXP = mybir.ActivationFunctionType.Exp
ALU = mybir.AluOpType

B, L, S, D = 2, 8192, 4096, 1024
H, HKV, HD, BS = 16, 4, 64, 16
HPG = H // HKV          # 4 q-heads per kv-group
NB = L // BS            # 512 blocks per batch
NBS = NB // 4           # 128 blocks per core
LS = NBS * BS           # 2048 tokens per core
SCALE_EFF = float(1.0 / (np.sqrt(HD) * BS))  # attn scale with /16 pooling folded in
NEG = 30000.0           # (mask-1)*NEG as exp bias kills masked columns

USE_F32R = False
NVP = S                 # kv positions processed (v1: full S, no compaction)
STRIP = 512             # enc rows per kv strip
NSTRIP = NVP // STRIP
NCH = NVP // 128        # 128-row score chunks


def _r(ap):
    return ap.bitcast(F32R) if USE_F32R else ap


def emit(nc, tc, ctx):
    x = nc.dram_tensor("x", [LS, D], F32, kind="ExternalInput").ap()
    enc = nc.dram_tensor("enc", [S, D], F32, kind="ExternalInput").ap()
    mask = nc.dram_tensor("mask", [S], I32, kind="ExternalInput").ap()
    wq = nc.dram_tensor("wq", [D, H * HD], F32, kind="ExternalInput").ap()
    wk = nc.dram_tensor("wk", [D, HKV * HD], F32, kind="ExternalInput").ap()
    wv = nc.dram_tensor("wv", [D, HKV * HD], F32, kind="ExternalInput").ap()
    wo = nc.dram_tensor("wo", [H * HD, D], F32, kind="ExternalInput").ap()
    out = nc.dram_tensor("out", [LS, D], F32, kind="ExternalOutput").ap()

    res = ctx.enter_context(tc.tile_pool(name="res", bufs=1))
    big = ctx.enter_context(tc.tile_pool(name="big", bufs=2))
    sm = ctx.enter_context(tc.tile_pool(name="sm", bufs=3))
    ps = ctx.enter_context(tc.tile_pool(name="ps", bufs=2, space="PSUM"))

    ident = res.tile([128, 128], F32, tag="ident")
    make_identity(nc, ident[:])
    # identity block on partitions 64-127 (for transposing base-64 slices)
    ident2 = res.tile([128, 64], F32, tag="ident2")
    nc.gpsimd.memset(ident2[:], 0.0)
    nc.gpsimd.affine_select(
        out=ident2[:],
        in_=ident2[:],
        compare_op=ALU.not_equal,
        fill=1.0,
        base=-64,
        pattern=[[-1, 64]],
        channel_multiplier=1,
    )

    # ---- mask -> per-chunk exp bias [128, NCH] ----
    mi = res.tile([128, NCH], I32, tag="mi")
    nc.sync.dma_start(mi[:], mask.rearrange("(c p) -> p c", p=128))
    mf = res.tile([128, NCH], F32, tag="mf")
    nc.vector.tensor_copy(mf[:], mi[:])
    biasb = res.tile([128, NCH], F32, tag="biasb")
    nc.vector.tensor_scalar(
        out=biasb[:], in0=mf[:], scalar1=1.0, scalar2=NEG,
        op0=ALU.subtract, op1=ALU.mult,
    )

    # ---- pooling: x [2048, D] -> pooled sum [128 blocks, D] ----
    pooled = res.tile([128, D], F32, tag="pooled")
    xv = x.rearrange("(p j) d -> p j d", j=BS)  # [128, 16, 1024]
    for r in range(4):
        xt = big.tile([128, 4 * D], F32, tag="xt")
        nc.sync.dma_start(
            xt[:].rearrange("p (j d) -> p j d", j=4), xv[:, 4 * r : 4 * r + 4, :]
        )
        h2 = big.tile([128, 2 * D], F32, tag="h2")
        nc.vector.tensor_add(h2[:], xt[:, 0 : 2 * D], xt[:, 2 * D : 4 * D])
        h1 = big.tile([128, D], F32, tag="h1")
        nc.vector.tensor_add(h1[:], h2[:, 0:D], h2[:, D : 2 * D])
        if r == 0:
            nc.vector.tensor_copy(pooled[:], h1[:])
        else:
            nc.vector.tensor_add(pooled[:], pooled[:], h1[:])

    # ---- q projection: q [128 blocks, H*HD] ----
    pooledT = res.tile([128, D], F32, tag="pooledT")  # 8 chunks of [128d, 128blk]
    for dc in range(8):
        tp = ps.tile([128, 128], F32, tag="tp")
        nc.tensor.transpose(tp[:], pooled[:, 128 * dc : 128 * (dc + 1)], ident[:])
        nc.vector.tensor_copy(pooledT[:, 128 * dc : 128 * (dc + 1)], tp[:])

    qps = [ps.tile([128, 512], F32, name=f"qps{i}", tag=f"mm{i}") for i in range(2)]
    for dc in range(8):
        wqt = big.tile([128, H * HD], F32, tag="wqt")
        nc.sync.dma_start(wqt[:], wq[128 * dc : 128 * (dc + 1), :])
        for half in range(2):
            nc.tensor.matmul(
                qps[half][:],
                lhsT=_r(pooledT[:, 128 * dc : 128 * (dc + 1)]),
                rhs=_r(wqt[:, 512 * half : 512 * (half + 1)]),
                start=(dc == 0),
                stop=(dc == 7),
            )
    qsb = res.tile([128, H * HD], F32, tag="qsb")
    for half in range(2):
        nc.vector.tensor_copy(qsb[:, 512 * half : 512 * (half + 1)], qps[half][:])

    # qT_g [64, HPG*128] per kv-group
    qT = [res.tile([64, HPG * 128], F32, name=f"qT{g}", tag=f"qT{g}") for g in range(HKV)]
    for g in range(HKV):
        for h in range(HPG):
            hh = g * HPG + h
            tp = ps.tile([128, 128], F32, tag="tp")
            nc.tensor.transpose(
                tp[0:64, 0:128], qsb[:, 64 * hh : 64 * (hh + 1)], ident[:]
            )
            nc.vector.tensor_copy(qT[g][:, 128 * h : 128 * (h + 1)], tp[0:64, 0:128])

    # ---- kv weights resident: wkv_g chunks [128 D, 64 k | 64 v] ----
    wkvt = []
    for g in range(HKV):
        row = []
        for dc in range(8):
            t = res.tile([128, 128], F32, name=f"wkv{g}_{dc}", tag=f"wkv{g}_{dc}")
            nc.sync.dma_start(
                t[:, 0:64], wk[128 * dc : 128 * (dc + 1), 64 * g : 64 * (g + 1)]
            )
            nc.sync.dma_start(
                t[:, 64:128], wv[128 * dc : 128 * (dc + 1), 64 * g : 64 * (g + 1)]
            )
            row.append(t)
        wkvt.append(row)

    # ---- main strip loop: enc -> encT -> kvT -> scores/exp/oT ----
    oTacc = [res.tile([65, 512], F32, name=f"oTacc{g}", tag=f"oTacc{g}") for g in range(HKV)]
    for st in range(NSTRIP):
        encT = [big.tile([128, STRIP], F32, name=f"encT{st}_{dc}", tag=f"encT{dc}") for dc in range(8)]
        for sc in range(STRIP // 128):
            et = big.tile([128, D], F32, tag="et")
            r0 = st * STRIP + sc * 128
            nc.sync.dma_start(et[:], enc[r0 : r0 + 128, :])
            for dc in range(8):
                tp = ps.tile([128, 128], F32, tag="tp")
                nc.tensor.transpose(tp[:], et[:, 128 * dc : 128 * (dc + 1)], ident[:])
                nc.vector.tensor_copy(encT[dc][:, 128 * sc : 128 * (sc + 1)], tp[:])
        for g in range(HKV):
            kvp = ps.tile([128, 512], F32, tag="mm0")
            for dc in range(8):
                nc.tensor.matmul(
                    kvp[:],
                    lhsT=_r(wkvt[g][dc][:]),
                    rhs=_r(encT[dc][:]),
                    start=(dc == 0),
                    stop=(dc == 7),
                )
            kvT = big.tile([128, STRIP], F32, tag="kvT")
            nc.vector.tensor_copy(kvT[:], kvp[:])
            for c in range(STRIP // 128):
                cg = st * (STRIP // 128) + c  # global chunk id
                sps = ps.tile([128, 512], F32, tag="mm1")
                nc.tensor.matmul(
                    sps[:],
                    lhsT=_r(kvT[0:64, 128 * c : 128 * (c + 1)]),
                    rhs=_r(qT[g][:]),
                    start=True,
                    stop=True,
                )
                pt = sm.tile([128, 512], F32, tag="pt")
                nc.scalar.activation(
                    pt[:], sps[:], EXP, bias=biasb[:, cg : cg + 1], scale=SCALE_EFF
                )
                vtp = ps.tile([128, 128], F32, tag="tp")
                nc.tensor.transpose(
                    vtp[0:128, 0:64],
                    kvT[64:128, 128 * c : 128 * (c + 1)],
                    ident2[64:128, 0:64],
                )
                va = sm.tile([128, 65], F32, tag="va")
                nc.vector.tensor_copy(va[:, 0:64], vtp[0:128, 0:64])
                nc.vector.memset(va[:, 64:65], 1.0)
                otp = ps.tile([65, 512], F32, tag="otp")
                nc.tensor.matmul(
                    otp[:], lhsT=_r(va[:]), rhs=_r(pt[:]), start=True, stop=True
                )
                if cg == 0:
                    nc.vector.tensor_copy(oTacc[g][:], otp[:])
                else:
                    nc.vector.tensor_add(oTacc[g][:], oTacc[g][:], otp[:])

    # ---- normalize: oTn_g = oTacc[0:64] * bcast(1/oTacc[64]) ----
    oTn = [res.tile([64, 512], F32, name=f"oTn{g}", tag=f"oTn{g}") for g in range(HKV)]
    ones64 = res.tile([1, 64], F32, tag="ones64")
    nc.vector.memset(ones64[:], 1.0)
    for g in range(HKV):
        rec = sm.tile([1, 512], F32, tag="rec")
        nc.vector.reciprocal(rec[:], oTacc[g][64:65, :])
        bc = ps.tile([64, 512], F32, tag="otp")
        nc.tensor.matmul(bc[:], lhsT=ones64[:], rhs=rec[:], start=True, stop=True)
        bcs = sm.tile([64, 512], F32, tag="bcs")
        nc.vector.tensor_copy(bcs[:], bc[:])
        nc.vector.tensor_tensor(
            out=oTn[g][:], in0=oTacc[g][0:64, :], in1=bcs[:], op=ALU.mult
        )

    # ---- output projection + broadcast write ----
    ops_ = [ps.tile([128, 512], F32, name=f"ops{i}", tag=f"mm{i}") for i in range(2)]
    for hh in range(H):
        g, h = hh // HPG, hh % HPG
        wot = sm.tile([64, D], F32, tag="wot")
        nc.sync.dma_start(wot[:], wo[64 * hh : 64 * (hh + 1), :])
        for half in range(2):
            nc.tensor.matmul(
                ops_[half][:],
                lhsT=_r(oTn[g][:, 128 * h : 128 * (h + 1)]),
                rhs=_r(wot[:, 512 * half : 512 * (half + 1)]),
                start=(hh == 0),
                stop=(hh == H - 1),
            )
    osb = res.tile([128, D], F32, tag="osb")
    for half in range(2):
        nc.vector.tensor_copy(osb[:, 512 * half : 512 * (half + 1)], ops_[half][:])
    ov = out.rearrange("(p j) d -> p j d", j=BS)
    for j in range(BS):
        nc.sync.dma_start(ov[:, j, :], osb[:])


_CACHE = {}


def _build():
    if "nc" not in _CACHE:
        from contextlib import ExitStack

        nc = bacc.Bacc("TRN2", target_bir_lowering=False, debug=False, num_devices=8)
        with tile.TileContext(nc) as tc, ExitStack() as ctx:
            emit(nc, tc, ctx)
        nc.compile()
        _CACHE["nc"] = nc
    return _CACHE["nc"]


def kernel(x, enc, mask, Wq, Wk, Wv, Wo):
    nc = _build()
    in_maps = []
    for core in range(8):
        b, r = core // 4, core % 4
        in_maps.append(
            {
                "x": np.ascontiguousarray(x[b, r * LS : (r + 1) * LS, :], np.float32),
                "enc": np.ascontiguousarray(enc[b], np.float32),
                "mask": np.ascontiguousarray(mask[b], np.int32),
                "wq": np.asarray(Wq, np.float32),
                "wk": np.asarray(Wk, np.float32),
                "wv": np.asarray(Wv, np.float32),
                "wo": np.asarray(Wo, np.float32),
            }
        )
    res = run_bass_kernel_spmd(nc, in_maps, core_ids=list(range(8)))
    out = np.empty((B, L, D), np.float32)
    for core in range(8):
        b, r = core // 4, core % 4
        out[b, r * LS : (r + 1) * LS, :] = res.results[core]["out"]
    return out


# revision 18
# speedup vs baseline: 1.5663x; 1.5663x over previous
"""Block cross-attention Trainium2 kernel (Bass/Tile), 8-core SPMD.

Reference computation (see problem statement):
  pooled = mean-pool x over blocks of 16 tokens        [B, nb, D]
  q = pooled @ Wq (16 heads), k/v = enc @ Wk/Wv (4 kv heads, GQA)
  p = softmax(q k^T * scale + mask)                    per kv-head group
  o = p @ v ; out = repeat(o @ Wo, 16 tokens/block)    [B, L, D]

Sharding: 8 cores = (batch b in {0,1}) x (block-range r in {0..3}).
Each core owns 128 query blocks (2048 tokens) of one batch and computes
ALL heads for them, so the output projection finishes on-device with no
cross-core reduction.  The kv projection (full S for all 4 kv heads) is
recomputed per core; softmax work is perfectly sharded.

Matmul inputs are float32r (TF32-like fast fp32): 1 cycle/row streaming
vs 4 for fp32.  The BIR verifier requires producers to emit f32r, so
SBUF tiles feeding matmuls are allocated as f32r.

Device pipeline per core:
  pool:   x-slice [2048, D] -> pooled-sum [128 blocks, D]  (DVE tree add;
          the /16 is folded into the exp scale)
  q:      pooledT (PE transpose) @ Wq -> q [128, 1024] -> per-head PE
          transposes -> qT_g [64, 4*128] per kv-group
  kv:     per 512-row strip of enc: PE transpose -> encT, kvT_g[128, NVP]
          resident = (Wk_g|Wv_g)^T @ encT
  attn:   per kv-group, per 128-row chunk: sT = kT^T @ qT_g, p =
          exp(scale*sT + maskbias) on ACT, v_aug = (v|1) via PE transpose,
          oT_g += v_aug^T @ p in PSUM (ones column accumulates the softmax
          denominator for free)
  norm:   oTn_g = oT_g[0:64] * bcast(1/oT_g[64])  (PE broadcast)
  outp:   out_blocks [128, D] = sum_h oTn^T @ Wo_h ; broadcast each block
          row to its 16 tokens on the DMA out.
"""

import numpy as np

import concourse.bass as bass
import concourse.tile as tile
from concourse import bacc, mybir
from concourse.bass_utils import run_bass_kernel_spmd
from concourse.masks import make_identity

F32 = mybir.dt.float32
F32R = mybir.dt.float32r
I32 = mybir.dt.int32
EXP = mybir.ActivationFunctionType.Exp
ALU = mybir.AluOpType

B, L, S, D = 2, 8192, 4096, 1024
H, HKV, HD, BS = 16, 4, 64, 16
HPG = H // HKV          # 4 q-heads per kv-group
NB = L // BS            # 512 blocks per batch
NBS = NB // 4           # 128 blocks per core
LS = NBS * BS           # 2048 tokens per core
SCALE_EFF = float(1.0 / (np.sqrt(HD) * BS))  # attn scale with /16 pooling folded in
NEG = 30000.0           # (mask-1)*NEG as exp bias kills masked columns

USE_F32R = True
DT = F32R if USE_F32R else F32
NVP = S                 # kv positions processed (v1: full S, no compaction)
STRIP = 512             # enc rows per kv strip
NSTRIP = NVP // STRIP
NCH = NVP // 128        # 128-row score chunks


def emit(nc, tc, ctx):
    x = nc.dram_tensor("x", [LS, D], F32, kind="ExternalInput").ap()
    enc = nc.dram_tensor("enc", [S, D], F32, kind="ExternalInput").ap()
    mask = nc.dram_tensor("mask", [S], I32, kind="ExternalInput").ap()
    wq = nc.dram_tensor("wq", [D, H * HD], F32, kind="ExternalInput").ap()
    wk = nc.dram_tensor("wk", [D, HKV * HD], F32, kind="ExternalInput").ap()
    wv = nc.dram_tensor("wv", [D, HKV * HD], F32, kind="ExternalInput").ap()
    wo = nc.dram_tensor("wo", [H * HD, D], F32, kind="ExternalInput").ap()
    out = nc.dram_tensor("out", [LS, D], F32, kind="ExternalOutput").ap()

    ctx.enter_context(nc.allow_low_precision(reason="f32r matmul inputs"))
    res = ctx.enter_context(tc.tile_pool(name="res", bufs=1))
    big = ctx.enter_context(tc.tile_pool(name="big", bufs=2))
    sm = ctx.enter_context(tc.tile_pool(name="sm", bufs=2))
    ps = ctx.enter_context(tc.tile_pool(name="ps", bufs=2, space="PSUM"))

    # f32r constants must be produced by a rounding op (DVE copy from f32)
    identf = res.tile([128, 128], F32, tag="identf")
    make_identity(nc, identf[:])
    ident = res.tile([128, 128], DT, tag="ident")
    nc.vector.tensor_copy(ident[:], identf[:])
    # identity block on partitions 64-127 (for transposing base-64 slices)
    ident2f = res.tile([128, 64], F32, tag="ident2f")
    nc.gpsimd.memset(ident2f[:], 0.0)
    nc.gpsimd.affine_select(
        out=ident2f[:],
        in_=ident2f[:],
        compare_op=ALU.not_equal,
        fill=1.0,
        base=-64,
        pattern=[[-1, 64]],
        channel_multiplier=1,
    )
    ident2 = res.tile([128, 64], DT, tag="ident2")
    nc.vector.tensor_copy(ident2[:], ident2f[:])
    onesf = res.tile([128, 1], F32, tag="onesf")
    nc.gpsimd.memset(onesf[:], 1.0)
    vones = res.tile([128, 1], DT, tag="vones")
    nc.vector.tensor_copy(vones[:], onesf[:])

    # ---- mask -> per-chunk exp bias [128, NCH] ----
    mi = res.tile([128, NCH], I32, tag="mi")
    nc.sync.dma_start(mi[:], mask.rearrange("(c p) -> p c", p=128))
    mf = res.tile([128, NCH], F32, tag="mf")
    nc.vector.tensor_copy(mf[:], mi[:])
    biasb = res.tile([128, NCH], F32, tag="biasb")
    nc.vector.tensor_scalar(
        out=biasb[:], in0=mf[:], scalar1=1.0, scalar2=NEG,
        op0=ALU.subtract, op1=ALU.mult,
    )

    # ---- pooling: x [2048, D] -> pooled sum [128 blocks, D] ----
    pooled = res.tile([128, D], DT, tag="pooled")
    acc = res.tile([128, D], F32, tag="acc")
    xv = x.rearrange("(p j) d -> p j d", j=BS)  # [128, 16, 1024]
    for r in range(8):
        xt = big.tile([128, 2 * D], F32, tag="xt")
        nc.sync.dma_start(
            xt[:].rearrange("p (j d) -> p j d", j=2), xv[:, 2 * r : 2 * r + 2, :]
        )
        h2 = big.tile([128, D], F32, tag="h2")
        nc.vector.tensor_add(h2[:], xt[:, 0:D], xt[:, D : 2 * D])
        if r == 0:
            nc.vector.tensor_copy(acc[:], h2[:])
        elif r < 7:
            nc.vector.tensor_add(acc[:], acc[:], h2[:])
        else:
            nc.vector.tensor_add(pooled[:], acc[:], h2[:])

    # ---- q projection: q [128 blocks, H*HD] ----
    pooledT = res.tile([128, D], DT, tag="pooledT")  # 8 chunks of [128d, 128blk]
    for dc in range(8):
        tp = ps.tile([128, 128], DT, tag="tp")
        nc.tensor.transpose(tp[:], pooled[:, 128 * dc : 128 * (dc + 1)], ident[:])
        nc.vector.tensor_copy(pooledT[:, 128 * dc : 128 * (dc + 1)], tp[:])

    qps = [ps.tile([128, 512], F32, name=f"qps{i}", tag=f"mm{i}") for i in range(2)]
    for dc in range(8):
        wqt = big.tile([128, H * HD], DT, tag="wqt")
        nc.sync.dma_start(wqt[:], wq[128 * dc : 128 * (dc + 1), :].bitcast(DT))
        for half in range(2):
            nc.tensor.matmul(
                qps[half][:],
                lhsT=pooledT[:, 128 * dc : 128 * (dc + 1)],
                rhs=wqt[:, 512 * half : 512 * (half + 1)],
                start=(dc == 0),
                stop=(dc == 7),
            )
    qsb = res.tile([128, H * HD], DT, tag="qsb")
    for half in range(2):
        nc.vector.tensor_copy(qsb[:, 512 * half : 512 * (half + 1)], qps[half][:])

    # qT_g [64, HPG*128] per kv-group
    qT = [
        res.tile([64, HPG * 128], DT, name=f"qT{g}", tag=f"qT{g}") for g in range(HKV)
    ]
    for g in range(HKV):
        for h in range(HPG):
            hh = g * HPG + h
            tp = ps.tile([128, 128], DT, tag="tp")
            nc.tensor.transpose(
                tp[0:64, 0:128], qsb[:, 64 * hh : 64 * (hh + 1)], ident[:]
            )
            nc.vector.tensor_copy(qT[g][:, 128 * h : 128 * (h + 1)], tp[0:64, 0:128])

    # ---- kv weights resident: wkv_g chunks [128 D, 64 k | 64 v] ----
    wkvt = []
    for g in range(HKV):
        row = []
        for dc in range(8):
            t = res.tile([128, 128], DT, name=f"wkv{g}_{dc}", tag=f"wkv{g}_{dc}")
            nc.sync.dma_start(
                t[:, 0:64], wk[128 * dc : 128 * (dc + 1), 64 * g : 64 * (g + 1)].bitcast(DT)
            )
            nc.sync.dma_start(
                t[:, 64:128], wv[128 * dc : 128 * (dc + 1), 64 * g : 64 * (g + 1)].bitcast(DT)
            )
            row.append(t)
        wkvt.append(row)

    # ---- kv projection: kvT_g [128, NVP] resident, per 512-col strip ----
    kvT = [
        res.tile([128, NVP], DT, name=f"kvT{g}", tag=f"kvT{g}") for g in range(HKV)
    ]
    for st in range(NSTRIP):
        encT = [
            big.tile([128, STRIP], DT, name=f"encT{st}_{dc}", tag=f"encT{dc}", bufs=1)
            for dc in range(8)
        ]
        for sc in range(STRIP // 128):
            et = big.tile([128, D], DT, tag="et")
            r0 = st * STRIP + sc * 128
            nc.sync.dma_start(et[:], enc[r0 : r0 + 128, :].bitcast(DT))
            for dc in range(8):
                tp = ps.tile([128, 128], DT, tag="tp")
                nc.tensor.transpose(tp[:], et[:, 128 * dc : 128 * (dc + 1)], ident[:])
                nc.vector.tensor_copy(encT[dc][:, 128 * sc : 128 * (sc + 1)], tp[:])
        for g in range(HKV):
            kvp = ps.tile([128, 512], F32, name=f"kvp{st}_{g}", tag="mm0")
            for dc in range(8):
                nc.tensor.matmul(
                    kvp[:],
                    lhsT=wkvt[g][dc][:],
                    rhs=encT[dc][:],
                    start=(dc == 0),
                    stop=(dc == 7),
                )
            nc.vector.tensor_copy(kvT[g][:, STRIP * st : STRIP * (st + 1)], kvp[:])

    # ---- attention: sT -> exp -> oT accumulated in PSUM ----
    oTn = [res.tile([64, 512], DT, name=f"oTn{g}", tag=f"oTn{g}") for g in range(HKV)]
    ones64 = res.tile([1, 64], DT, tag="ones64")
    nc.vector.tensor_copy(ones64[:], onesf[0:1, 0:1].to_broadcast([1, 64]))
    for g in range(HKV):
        otp = ps.tile([65, 512], F32, name=f"otp{g}", tag="otp")
        for c in range(NCH):
            sps = ps.tile([128, 512], F32, name=f"sps{g}_{c}", tag="mm1")
            nc.tensor.matmul(
                sps[:],
                lhsT=kvT[g][0:64, 128 * c : 128 * (c + 1)],
                rhs=qT[g][:],
                start=True,
                stop=True,
            )
            pt = sm.tile([128, 512], DT, tag="pt")
            nc.scalar.activation(
                pt[:], sps[:], EXP, bias=biasb[:, c : c + 1], scale=SCALE_EFF
            )
            vtp = ps.tile([128, 128], DT, tag="tp")
            nc.tensor.transpose(
                vtp[0:128, 0:64],
                kvT[g][64:128, 128 * c : 128 * (c + 1)],
                ident2[64:128, 0:64],
            )
            va = sm.tile([128, 65], DT, tag="va")
            nc.vector.tensor_copy(va[:, 0:64], vtp[0:128, 0:64])
            nc.vector.tensor_copy(va[:, 64:65], vones[:])
            nc.tensor.matmul(
                otp[:], lhsT=va[:], rhs=pt[:], start=(c == 0), stop=(c == NCH - 1)
            )
        # normalize: oTn_g = otp[0:64] * bcast(1/otp[64])
        rec = sm.tile([1, 512], DT, tag="rec")
        nc.vector.reciprocal(rec[:], otp[64:65, :])
        bc = ps.tile([64, 512], F32, name=f"bc{g}", tag="mm0")
        nc.tensor.matmul(bc[:], lhsT=ones64[:], rhs=rec[:], start=True, stop=True)
        bcs = sm.tile([64, 512], F32, tag="bcs")
        nc.vector.tensor_copy(bcs[:], bc[:])
        nc.vector.tensor_tensor(
            out=oTn[g][:], in0=otp[0:64, :], in1=bcs[:], op=ALU.mult
        )

    # ---- output projection + broadcast write ----
    ops_ = [ps.tile([128, 512], F32, name=f"ops{i}", tag=f"mm{i}") for i in range(2)]
    for hh in range(H):
        g, h = hh // HPG, hh % HPG
        wot = sm.tile([64, D], DT, tag="wot")
        nc.sync.dma_start(wot[:], wo[64 * hh : 64 * (hh + 1), :].bitcast(DT))
        for half in range(2):
            nc.tensor.matmul(
                ops_[half][:],
                lhsT=oTn[g][:, 128 * h : 128 * (h + 1)],
                rhs=wot[:, 512 * half : 512 * (half + 1)],
                start=(hh == 0),
                stop=(hh == H - 1),
            )
    osb = res.tile([128, D], F32, tag="osb")
    for half in range(2):
        nc.vector.tensor_copy(osb[:, 512 * half : 512 * (half + 1)], ops_[half][:])
    ov = out.rearrange("(p j) d -> p j d", j=BS)
    for j in range(BS):
        nc.sync.dma_start(ov[:, j, :], osb[:])


_CACHE = {}


def _build():
    if "nc" not in _CACHE:
        from contextlib import ExitStack

        nc = bacc.Bacc("TRN2", target_bir_lowering=False, debug=False, num_devices=8)
        with tile.TileContext(nc) as tc, ExitStack() as ctx:
            emit(nc, tc, ctx)
        nc.compile()
        _CACHE["nc"] = nc
    return _CACHE["nc"]


def kernel(x, enc, mask, Wq, Wk, Wv, Wo):
    nc = _build()
    in_maps = []
    for core in range(8):
        b, r = core // 4, core % 4
        in_maps.append(
            {
                "x": np.ascontiguousarray(x[b, r * LS : (r + 1) * LS, :], np.float32),
                "enc": np.ascontiguousarray(enc[b], np.float32),
                "mask": np.ascontiguousarray(mask[b], np.int32),
                "wq": np.asarray(Wq, np.float32),
                "wk": np.asarray(Wk, np.float32),
                "wv": np.asarray(Wv, np.float32),
                "wo": np.asarray(Wo, np.float32),
            }
        )
    res = run_bass_kernel_spmd(nc, in_maps, core_ids=list(range(8)))
    out = np.empty((B, L, D), np.float32)
    for core in range(8):
        b, r = core // 4, core % 4
        out[b, r * LS : (r + 1) * LS, :] = res.results[core]["out"]
    return out
